# revision 1
# baseline (speedup 1.0000x reference)
"""Trainium2 Bass kernel for the SE-gated Non-local block (rank-1 attention).

Math (per batch item b, x viewed as [C, N] with N = H*W):
    S[c]    = sum_n x[c, n]                      (spatial sum)
    hid     = relu((se_w1 / N) @ S + se_b1)      (SE bottleneck; 1/N folds the mean)
    gate    = sigmoid(se_w2 @ hid + se_b2)       [C]
    w3e     = gate * [g_w | theta_w | phi_w]     [C, 3]   (gate folded into projections)
    proj    = w3e.T @ x + [g_b, theta_b, phi_b]  [3, N]   (rows: g, theta, phi)
    s_raw   = sum_n proj[0] * proj[2]
    out     = x + (A * s_raw) (outer) theta + Bc (outer) ones   where
              inv = bn_gamma / sqrt(bn_var + eps)
              A   = W_w * inv / N                (1/N folds the f/N normalizer)
              Bc  = (W_b - bn_mean) * inv + bn_beta

Precision split: the correction term A*s*theta has rms ~5e-6 vs |x| ~ 1, so the
whole gate/projection path runs in bf16 (costs ~1e-8 output rel err); x and the
Bc affine stay exact f32.

Sharding: pure data parallel, 2 of the 16 batch items per core, params
replicated, no collectives.  Each batch item's x ([512, 4608] f32, 9.4 MB)
stays resident in SBUF between the stats pass and the output pass, so HBM
traffic is near the minimum read-x + write-out (37.7 MB/core ~ 105 us).

Engine budget per core: DMA ~105 us (bound), ACT ~70 us (bf16 cast+rowsum,
affine), DVE ~45 us (psum copies, final adds), PE ~45 us (bf16 proj matmuls).
"""

import numpy as np

B, C, H, W = 16, 512, 96, 48
N = H * W            # 4608
P = 128
KC = C // P          # 4 channel chunks
NB = 512             # free-dim block = one fp32 PSUM bank
NJ = N // NB         # 9
NCORES = 8
BPC = B // NCORES    # 2 batch items per core
SE_C = C // 16       # 32
BN_EPS = 1e-5

_CACHE = {}
LAST_RESULTS = None


def _build_bass(xpool_bufs=4, stage="full"):
    # stage: bisection aid — "loads", "se", "proj", "rows", "full"
    S = {"loads": 0, "se": 1, "proj": 2, "rows": 3, "full": 4}[stage]
    import concourse.mybir as mybir
    from concourse.bacc import Bacc
    from concourse.tile import TileContext

    f32 = mybir.dt.float32
    bf16 = mybir.dt.bfloat16
    AF = mybir.ActivationFunctionType
    AX = mybir.AxisListType
    ALU = mybir.AluOpType

    nc = Bacc()
    xs = nc.dram_tensor("xs", [BPC, C, N], f32, kind="ExternalInput")
    w1 = nc.dram_tensor("w1", [P, KC, SE_C], f32, kind="ExternalInput")
    w2 = nc.dram_tensor("w2", [SE_C, C], f32, kind="ExternalInput")
    b1 = nc.dram_tensor("b1", [SE_C, 1], f32, kind="ExternalInput")
    b2 = nc.dram_tensor("b2", [P, KC], f32, kind="ExternalInput")
    w3 = nc.dram_tensor("w3", [P, KC, 3], bf16, kind="ExternalInput")
    pb = nc.dram_tensor("pb", [3, 1], f32, kind="ExternalInput")
    at = nc.dram_tensor("at", [P, KC], f32, kind="ExternalInput")   # A chunks
    bc = nc.dram_tensor("bc", [P, KC], f32, kind="ExternalInput")   # Bc chunks
    out_d = nc.dram_tensor("out", [BPC, C, N], f32, kind="ExternalOutput")
    # per-batch DRAM scratch for the g/theta/phi rows (partitions 1/2 of proj
    # are unreachable by compute engines; bounce through HBM to partition 0 /
    # broadcast / a 128-partition reshape)
    tp_scr = nc.dram_tensor("tp_scr", [BPC, 3, N], bf16)

    MR = N // P  # 36: elements per partition in the reshaped g/phi rows
    HB = N // 2  # half-chunk for the output-phase affine+add pipeline

    with TileContext(nc) as tc:
        with (
            tc.tile_pool(name="wpool", bufs=1) as wpool,
            tc.tile_pool(name="xpool", bufs=xpool_bufs) as xpool,
            tc.tile_pool(name="bpool", bufs=4) as bpool,
            tc.tile_pool(name="ppool", bufs=1) as ppool,
            tc.tile_pool(name="spool", bufs=2) as spool,
            tc.tile_pool(name="tpool", bufs=3) as tpool,
            tc.tile_pool(name="ps_se", bufs=2, space="PSUM") as ps_se,
            tc.tile_pool(name="ps_pp", bufs=3, space="PSUM") as ps_pp,
        ):
            w1t = wpool.tile([P, KC, SE_C], f32, tag="w1t")
            w2t = wpool.tile([SE_C, C], f32, tag="w2t")
            b1t = wpool.tile([SE_C, 1], f32, tag="b1t")
            b2t = wpool.tile([P, KC], f32, tag="b2t")
            w3t = wpool.tile([P, KC, 3], bf16, tag="w3t")
            pbt = wpool.tile([3, 1], f32, tag="pbt")
            att = wpool.tile([P, KC], f32, tag="att")
            bct = wpool.tile([P, KC], f32, tag="bct")
            on128 = wpool.tile([P, P], f32, tag="on128")  # all-ones (part. sum)
            ubt = wpool.tile([P, N], bf16, tag="ubt")     # theta bcast to 128p

            nc.gpsimd.memset(on128[:], 1.0)
            for t, d in ((w1t, w1), (w2t, w2), (b1t, b1), (b2t, b2),
                         (w3t, w3), (pbt, pb), (att, at), (bct, bc)):
                nc.gpsimd.dma_start(out=t[:], in_=d[:])

            for b in range(BPC):
                # ---- load x (f32, resident for the whole batch item) ----
                xts = []
                for k in range(KC):
                    xt = xpool.tile([P, N], f32, tag="xt")
                    nc.sync.dma_start(out=xt[:], in_=xs[b, k * P:(k + 1) * P, :])
                    xts.append(xt)

                # ---- bf16 working copy of x + spatial sums (one ACT op) ----
                xbs = []
                xp = spool.tile([P, KC], f32, tag="xp")
                for k in range(KC):
                    xb = bpool.tile([P, N], bf16, tag="xb")
                    nc.scalar.activation(out=xb[:], in_=xts[k][:],
                                         func=AF.Identity,
                                         accum_out=xp[:, k:k + 1])
                    xbs.append(xb)

                if S >= 1:
                    # ---- SE gate ----
                    php = ps_se.tile([SE_C, 1], f32, tag="ps_se")
                    for k in range(KC):
                        nc.tensor.matmul(php[:], w1t[:, k, :], xp[:, k:k + 1],
                                         start=(k == 0), stop=(k == KC - 1))
                    hid = spool.tile([SE_C, 1], f32, tag="hid")
                    nc.scalar.activation(out=hid[:], in_=php[:], func=AF.Relu,
                                         bias=b1t[:], scale=1.0)

                    gate = spool.tile([P, KC], f32, tag="gate")
                    for k in range(KC):
                        gp = ps_se.tile([P, 1], f32, tag="ps_se")
                        nc.tensor.matmul(gp[:], w2t[:, k * P:(k + 1) * P], hid[:],
                                         start=True, stop=True)
                        nc.scalar.activation(out=gate[:, k:k + 1], in_=gp[:],
                                             func=AF.Sigmoid, bias=b2t[:, k:k + 1],
                                             scale=1.0)

                if S >= 2:
                    # ---- gated projections: proj = w3e.T @ x (bf16 PE) ----
                    w3e = spool.tile([P, KC, 3], bf16, tag="w3e")
                    for k in range(KC):
                        nc.vector.tensor_scalar_mul(out=w3e[:, k, :],
                                                    in0=w3t[:, k, :],
                                                    scalar1=gate[:, k:k + 1])

                    proj = ppool.tile([3, N], bf16, tag="proj")
                    for j in range(NJ):
                        pp = ps_pp.tile([3, NB], f32, tag="pp")
                        for k in range(KC):
                            nc.tensor.matmul(pp[:], w3e[:, k, :],
                                             xbs[k][:, j * NB:(j + 1) * NB],
                                             start=(k == 0), stop=(k == KC - 1))
                        nc.vector.tensor_scalar_add(
                            out=proj[:, j * NB:(j + 1) * NB],
                            in0=pp[:], scalar1=pbt[:])

                if S >= 3:
                    # ---- g/theta/phi rows -> HBM; theta back broadcast to all
                    #      128 partitions, g and phi back reshaped [128, 36] so
                    #      the dot product uses every DVE lane ----
                    nc.gpsimd.dma_start(out=tp_scr[b], in_=proj[:])
                    nc.sync.dma_start(
                        out=ubt[:],
                        in_=tp_scr[b, 1:2, :].partition_broadcast(P)[:, 0, :])
                    g_rs = spool.tile([P, MR], bf16, tag="g_rs")
                    p_rs = spool.tile([P, MR], bf16, tag="p_rs")
                    nc.sync.dma_start(
                        out=g_rs[:],
                        in_=tp_scr[b, 0, :].rearrange("(p m) -> p m", p=P))
                    nc.sync.dma_start(
                        out=p_rs[:],
                        in_=tp_scr[b, 2, :].rearrange("(p m) -> p m", p=P))
                    # ---- s_raw = <g, phi>: elementwise mul, per-partition
                    #      reduce, then cross-partition sum via ones matmul ----
                    prod = spool.tile([P, MR], f32, tag="prod")
                    nc.vector.tensor_mul(out=prod[:], in0=g_rs[:], in1=p_rs[:])
                    r1 = spool.tile([P, 1], f32, tag="r1")
                    nc.vector.reduce_sum(out=r1[:], in_=prod[:], axis=AX.X)
                    sb = ps_se.tile([P, 1], f32, tag="ps_se")
                    nc.tensor.matmul(sb[:], on128[:], r1[:], start=True,
                                     stop=True)
                    ast = spool.tile([P, KC], f32, tag="ast")
                    nc.vector.tensor_scalar_mul(out=ast[:], in0=att[:],
                                                scalar1=sb[:])

                # ---- out = x + (A*s) * theta_bcast + Bc, store ----
                # halves alternate engines: ACT-affine/DVE-add on even halves,
                # DVE-affine/GpSimd-add on odd — balances the three engines
                for k in range(KC):
                    if S >= 4:
                        for h in range(2):
                            sl = slice(h * HB, (h + 1) * HB)
                            t1 = tpool.tile([P, HB], f32, tag="t1")
                            if h == 0:
                                nc.scalar.activation(out=t1[:], in_=ubt[:, sl],
                                                     func=AF.Identity,
                                                     scale=ast[:, k:k + 1],
                                                     bias=bct[:, k:k + 1])
                                nc.vector.tensor_add(out=xts[k][:, sl],
                                                     in0=xts[k][:, sl],
                                                     in1=t1[:])
                            else:
                                nc.vector.tensor_scalar(
                                    out=t1[:], in0=ubt[:, sl],
                                    scalar1=ast[:, k:k + 1],
                                    scalar2=bct[:, k:k + 1],
                                    op0=ALU.mult, op1=ALU.add)
                                nc.gpsimd.tensor_add(out=xts[k][:, sl],
                                                     in0=xts[k][:, sl],
                                                     in1=t1[:])
                    nc.scalar.dma_start(out=out_d[b, k * P:(k + 1) * P, :],
                                        in_=xts[k][:])

    nc.finalize()  # runs Bacc compile passes (wait splitting, reg alloc, ...)
    return nc


def kernel(**inputs):
    global LAST_RESULTS
    from concourse.bass_utils import run_bass_kernel_spmd

    a = {k: np.asarray(v, dtype=np.float32) for k, v in inputs.items()}
    x = np.ascontiguousarray(a["x"]).reshape(B, C, N)

    inv = a["bn_gamma"] / np.sqrt(a["bn_var"] + BN_EPS)
    A = (a["W_w"] * inv / N).astype(np.float32)
    Bc = ((a["W_b"] - a["bn_mean"]) * inv + a["bn_beta"]).astype(np.float32)

    w1h = np.ascontiguousarray(
        (a["se_w1"] / N).T.reshape(KC, P, SE_C).transpose(1, 0, 2)).astype(np.float32)
    w2h = np.ascontiguousarray(a["se_w2"].T).astype(np.float32)
    b1h = np.ascontiguousarray(a["se_b1"].reshape(SE_C, 1))
    b2h = np.ascontiguousarray(a["se_b2"].reshape(KC, P).T)
    import ml_dtypes
    w3h = np.ascontiguousarray(
        np.stack([a["g_w"], a["theta_w"], a["phi_w"]], axis=1)
        .reshape(KC, P, 3).transpose(1, 0, 2)).astype(ml_dtypes.bfloat16)
    pbh = np.array([[a["g_b"]], [a["theta_b"]], [a["phi_b"]]], dtype=np.float32)
    ath = np.ascontiguousarray(A.reshape(KC, P).T)
    bch = np.ascontiguousarray(Bc.reshape(KC, P).T)

    if "nc" not in _CACHE:
        _CACHE["nc"] = _build_bass()
    nc = _CACHE["nc"]

    in_maps = []
    for c in range(NCORES):
        in_maps.append({
            "xs": np.ascontiguousarray(x[c * BPC:(c + 1) * BPC]),
            "w1": w1h, "w2": w2h, "b1": b1h, "b2": b2h,
            "w3": w3h, "pb": pbh, "at": ath, "bc": bch,
        })

    res = run_bass_kernel_spmd(nc, in_maps, core_ids=list(range(NCORES)))
    LAST_RESULTS = res

    out = np.concatenate([res.results[c]["out"] for c in range(NCORES)], axis=0)
    return np.ascontiguousarray(out.reshape(B, C, H, W))



# revision 17
# speedup vs baseline: 1.5169x; 1.5169x over previous
"""Trainium2 Bass kernel for the SE-gated Non-local block (rank-1 attention).

Math (per batch item b, x viewed as [C, N] with N = H*W):
    S[c]    = sum_n x[c, n]                      (spatial sum)
    hid     = relu((se_w1 / N) @ S + se_b1)      (SE bottleneck; 1/N folds the mean)
    gate    = sigmoid(se_w2 @ hid + se_b2)       [C]
    w3e     = gate * [g_w | theta_w | phi_w]     [C, 3]   (gate folded into projections)
    proj    = w3e.T @ x + [g_b, theta_b, phi_b]  [3, N]   (rows: g, theta, phi)
    s_raw   = sum_n proj[0] * proj[2]
    out     = x + (A * s_raw) (outer) theta + Bc (outer) ones   where
              inv = bn_gamma / sqrt(bn_var + eps)
              A   = W_w * inv / N                (1/N folds the f/N normalizer)
              Bc  = (W_b - bn_mean) * inv + bn_beta

Memory-roofline design: the only mandatory HBM traffic is read-x + write-out
(37.7 MB/core at 358 GB/s ~ 105 us).  To keep the DMA rings saturated the
whole time, x is held in SBUF as *bf16* (4.7 MB/item instead of 9.4), so both
batch items of a core fit on-chip and all 8 chunk loads stream back-to-back
while item 0's gate/projection chain overlaps item 1's loads, and item 0's
stores overlap item 1's chain.  The f32->bf16 cast happens inside the load
DMA (SWDGE) and the bf16->f32 cast inside the store DMA, so no engine pass
touches the bulk data except the single in-place correction add.

Precision: out = bf16(x) + correction, quantized to bf16 before the store
cast.  That costs ~1.2e-3 output rel err (bf16 mantissa), far inside the
2e-2 gate; the correction term itself (rms ~5e-6 vs |x| ~ 1) runs in bf16
end-to-end as before.

Queue layout: x loads + out stores on the gpsimd SWDGE ring (they need the
dtype cast), weight loads + the g/theta/phi row shuffles on the sync HWDGE
ring, so neither blocks the other.  Sharding: pure data parallel, 2 of the
16 batch items per core, params replicated, no collectives.
"""

import numpy as np

B, C, H, W = 16, 512, 96, 48
N = H * W            # 4608
P = 128
KC = C // P          # 4 channel chunks
NB = 512             # free-dim block = one fp32 PSUM bank
NJ = N // NB         # 9
NCORES = 8
BPC = B // NCORES    # 2 batch items per core
SE_C = C // 16       # 32
BN_EPS = 1e-5

_CACHE = {}
LAST_RESULTS = None


def _build_bass():
    # g,phi rows bounce through a DRAM scratch ([3,N] write + two [128,36]
    # reshaped reads; an SBUF->SBUF reshape is not expressible -- the BIR
    # verifier requires partition dims to index real partitions).  theta's
    # broadcast to all 128 partitions is on-chip (selector matmul on PE).
    import concourse.mybir as mybir
    from concourse.bacc import Bacc
    from concourse.tile import TileContext

    f32 = mybir.dt.float32
    bf16 = mybir.dt.bfloat16
    AF = mybir.ActivationFunctionType
    AX = mybir.AxisListType
    ALU = mybir.AluOpType

    nc = Bacc()
    xs = nc.dram_tensor("xs", [BPC, C, N], f32, kind="ExternalInput")
    w1 = nc.dram_tensor("w1", [P, KC, SE_C], f32, kind="ExternalInput")
    w2 = nc.dram_tensor("w2", [SE_C, C], f32, kind="ExternalInput")
    b1 = nc.dram_tensor("b1", [SE_C, 1], f32, kind="ExternalInput")
    b2 = nc.dram_tensor("b2", [P, KC], f32, kind="ExternalInput")
    w3 = nc.dram_tensor("w3", [P, KC, 3], bf16, kind="ExternalInput")
    pb = nc.dram_tensor("pb", [3, 1], f32, kind="ExternalInput")
    at = nc.dram_tensor("at", [P, KC], f32, kind="ExternalInput")   # A chunks
    bc = nc.dram_tensor("bc", [P, KC], f32, kind="ExternalInput")   # Bc chunks
    sel = nc.dram_tensor("sel", [3, P], bf16, kind="ExternalInput")  # theta row
    out_d = nc.dram_tensor("out", [BPC, C, N], f32, kind="ExternalOutput")
    tp_scr = nc.dram_tensor("tp_scr", [BPC, 3, N], bf16)

    MR = N // P  # 36: elements per partition in the reshaped g/phi rows

    with TileContext(nc) as tc:
        with (
            tc.tile_pool(name="wpool", bufs=1) as wpool,
            tc.tile_pool(name="xpool", bufs=2 * KC) as xpool,
            tc.tile_pool(name="ppool", bufs=2) as ppool,
            tc.tile_pool(name="spool", bufs=2) as spool,
            tc.tile_pool(name="tpool", bufs=2) as tpool,
            tc.tile_pool(name="ps_se", bufs=2, space="PSUM") as ps_se,
            tc.tile_pool(name="ps_pp", bufs=3, space="PSUM") as ps_pp,
            tc.tile_pool(name="ps_ub", bufs=2, space="PSUM") as ps_ub,
        ):
            w1t = wpool.tile([P, KC, SE_C], f32, tag="w1t")
            w2t = wpool.tile([SE_C, C], f32, tag="w2t")
            b1t = wpool.tile([SE_C, 1], f32, tag="b1t")
            b2t = wpool.tile([P, KC], f32, tag="b2t")
            w3t = wpool.tile([P, KC, 3], bf16, tag="w3t")
            pbt = wpool.tile([3, 1], f32, tag="pbt")
            att = wpool.tile([P, KC], f32, tag="att")
            bct = wpool.tile([P, KC], f32, tag="bct")
            selt = wpool.tile([3, P], bf16, tag="selt")
            on128 = wpool.tile([P, P], f32, tag="on128")  # all-ones (part. sum)

            # weights on the sync ring so the gpsimd ring is free for x loads
            nc.vector.memset(on128[:], 1.0)
            for t, d in ((w1t, w1), (w2t, w2), (b1t, b1), (b2t, b2),
                         (w3t, w3), (pbt, pb), (att, at), (bct, bc),
                         (selt, sel)):
                nc.sync.dma_start(out=t[:], in_=d[:])

            # ---- all 8 x-chunk loads enqueued upfront (SWDGE f32->bf16) ----
            xbs = [[None] * KC for _ in range(BPC)]
            for b in range(BPC):
                for k in range(KC):
                    xb = xpool.tile([P, N], bf16, tag="xb")
                    nc.gpsimd.dma_start(out=xb[:], in_=xs[b, k * P:(k + 1) * P, :])
                    xbs[b][k] = xb

            for b in range(BPC):
                # ---- spatial sums: in-place *1.0 tensor_scalar (4x DVE
                #      mode) with the per-partition sum as accum rider;
                #      reduce_sum would run at 1x (4.8us/chunk) ----
                xp = spool.tile([P, KC], f32, tag="xp")
                for k in range(KC):
                    nc.vector.tensor_scalar(out=xbs[b][k][:], in0=xbs[b][k][:],
                                            scalar1=1.0, scalar2=0.0,
                                            op0=ALU.mult, op1=ALU.add,
                                            accum_out=xp[:, k:k + 1])

                # ---- SE gate ----
                php = ps_se.tile([SE_C, 1], f32, tag="ps_se")
                for k in range(KC):
                    nc.tensor.matmul(php[:], w1t[:, k, :], xp[:, k:k + 1],
                                     start=(k == 0), stop=(k == KC - 1))
                hid = spool.tile([SE_C, 1], f32, tag="hid")
                nc.scalar.activation(out=hid[:], in_=php[:], func=AF.Relu,
                                     bias=b1t[:], scale=1.0)

                gate = spool.tile([P, KC], f32, tag="gate")
                for k in range(KC):
                    gp = ps_se.tile([P, 1], f32, tag="ps_se")
                    nc.tensor.matmul(gp[:], w2t[:, k * P:(k + 1) * P], hid[:],
                                     start=True, stop=True)
                    nc.scalar.activation(out=gate[:, k:k + 1], in_=gp[:],
                                         func=AF.Sigmoid, bias=b2t[:, k:k + 1],
                                         scale=1.0)

                # ---- gated projections: proj = w3e.T @ x (bf16 PE) ----
                w3e = spool.tile([P, KC, 3], bf16, tag="w3e")
                for k in range(KC):
                    nc.vector.tensor_scalar_mul(out=w3e[:, k, :],
                                                in0=w3t[:, k, :],
                                                scalar1=gate[:, k:k + 1])

                proj = ppool.tile([3, N], bf16, tag="proj")
                ubt = tpool.tile([P, N], bf16, tag="ubt")
                for j in range(NJ):
                    pp = ps_pp.tile([3, NB], f32, tag="pp")
                    for k in range(KC):
                        nc.tensor.matmul(pp[:], w3e[:, k, :],
                                         xbs[b][k][:, j * NB:(j + 1) * NB],
                                         start=(k == 0), stop=(k == KC - 1))
                    nc.scalar.activation(out=proj[:, j * NB:(j + 1) * NB],
                                         in_=pp[:], func=AF.Identity,
                                         bias=pbt[:], scale=1.0)
                    # theta -> all 128 partitions, on-chip: selector matmul
                    ub_ps = ps_ub.tile([P, NB], f32, tag="ub_ps")
                    nc.tensor.matmul(ub_ps[:], selt[:],
                                     proj[:, j * NB:(j + 1) * NB],
                                     start=True, stop=True)
                    nc.scalar.activation(out=ubt[:, j * NB:(j + 1) * NB],
                                         in_=ub_ps[:], func=AF.Identity,
                                         scale=1.0)

                # ---- g,phi -> [128, 36] (DRAM bounce) so the dot product
                #      uses every DVE lane ----
                g_rs = spool.tile([P, MR], bf16, tag="g_rs")
                p_rs = spool.tile([P, MR], bf16, tag="p_rs")
                nc.sync.dma_start(out=tp_scr[b], in_=proj[:])
                nc.sync.dma_start(
                    out=g_rs[:],
                    in_=tp_scr[b, 0, :].rearrange("(p m) -> p m", p=P))
                nc.sync.dma_start(
                    out=p_rs[:],
                    in_=tp_scr[b, 2, :].rearrange("(p m) -> p m", p=P))

                # ---- s_raw = <g, phi>; ast = A * s_raw ----
                prod = spool.tile([P, MR], f32, tag="prod")
                nc.vector.tensor_mul(out=prod[:], in0=g_rs[:], in1=p_rs[:])
                r1 = spool.tile([P, 1], f32, tag="r1")
                nc.vector.reduce_sum(out=r1[:], in_=prod[:], axis=AX.X)
                sb = ps_se.tile([P, 1], f32, tag="ps_se")
                nc.tensor.matmul(sb[:], on128[:], r1[:], start=True, stop=True)
                ast = spool.tile([P, KC], f32, tag="ast")
                nc.vector.tensor_scalar_mul(out=ast[:], in0=att[:],
                                            scalar1=sb[:])

                # ---- out = bf16(x) + (A*s)*theta + Bc, in place; store with
                #      bf16->f32 cast in the DMA.  Affines alternate DVE
                #      (4x mode) / ACT to balance; adds are 2x DVE ----
                for k in range(KC):
                    t1 = tpool.tile([P, N], bf16, tag="t1")
                    if k % 2 == 0:
                        nc.vector.tensor_scalar(out=t1[:], in0=ubt[:],
                                                scalar1=ast[:, k:k + 1],
                                                scalar2=bct[:, k:k + 1],
                                                op0=ALU.mult, op1=ALU.add)
                    else:
                        nc.scalar.activation(out=t1[:], in_=ubt[:],
                                             func=AF.Identity,
                                             scale=ast[:, k:k + 1],
                                             bias=bct[:, k:k + 1])
                    nc.vector.tensor_add(out=xbs[b][k][:],
                                         in0=xbs[b][k][:], in1=t1[:])
                    nc.gpsimd.dma_start(out=out_d[b, k * P:(k + 1) * P, :],
                                        in_=xbs[b][k][:])

    nc.finalize()  # runs Bacc compile passes (wait splitting, reg alloc, ...)
    return nc


def kernel(**inputs):
    global LAST_RESULTS
    from concourse.bass_utils import run_bass_kernel_spmd

    a = {k: np.asarray(v, dtype=np.float32) for k, v in inputs.items()}
    x = np.ascontiguousarray(a["x"]).reshape(B, C, N)

    inv = a["bn_gamma"] / np.sqrt(a["bn_var"] + BN_EPS)
    A = (a["W_w"] * inv / N).astype(np.float32)
    Bc = ((a["W_b"] - a["bn_mean"]) * inv + a["bn_beta"]).astype(np.float32)

    w1h = np.ascontiguousarray(
        (a["se_w1"] / N).T.reshape(KC, P, SE_C).transpose(1, 0, 2)).astype(np.float32)
    w2h = np.ascontiguousarray(a["se_w2"].T).astype(np.float32)
    b1h = np.ascontiguousarray(a["se_b1"].reshape(SE_C, 1))
    b2h = np.ascontiguousarray(a["se_b2"].reshape(KC, P).T)
    import ml_dtypes
    w3h = np.ascontiguousarray(
        np.stack([a["g_w"], a["theta_w"], a["phi_w"]], axis=1)
        .reshape(KC, P, 3).transpose(1, 0, 2)).astype(ml_dtypes.bfloat16)
    pbh = np.array([[a["g_b"]], [a["theta_b"]], [a["phi_b"]]], dtype=np.float32)
    ath = np.ascontiguousarray(A.reshape(KC, P).T)
    bch = np.ascontiguousarray(Bc.reshape(KC, P).T)
    selh = np.zeros((3, P), dtype=ml_dtypes.bfloat16)
    selh[1, :] = 1.0

    if "nc" not in _CACHE:
        _CACHE["nc"] = _build_bass()
    nc = _CACHE["nc"]

    in_maps = []
    for c in range(NCORES):
        in_maps.append({
            "xs": np.ascontiguousarray(x[c * BPC:(c + 1) * BPC]),
            "w1": w1h, "w2": w2h, "b1": b1h, "b2": b2h,
            "w3": w3h, "pb": pbh, "at": ath, "bc": bch, "sel": selh,
        })

    res = run_bass_kernel_spmd(nc, in_maps, core_ids=list(range(NCORES)))
    LAST_RESULTS = res

    out = np.concatenate([res.results[c]["out"] for c in range(NCORES)], axis=0)
    return np.ascontiguousarray(out.reshape(B, C, H, W))
